# revision 14
# baseline (speedup 1.0000x reference)
"""Causal single-head attention on 8 TRN2 NeuronCores.

Problem: K,Q,V [4, 4096, 1024] f32, Wk/Wq/Wv [1024, 64] f32.
out[b,q,:] = softmax_causal((Q Wq)(K Wk)^T / 8) @ (V Wv)

Sharding: core c = 2b+h owns batch b = c//2, half h = c%2. Each batch's
4096 queries are split into 8 tiers of 512; tier t (1-based) attends to a
key prefix rounded up to 512*t. Each tier's 512 queries are split in half
(256 each) across the batch's two cores, so every core executes an
IDENTICAL instruction stream: 8 attention instances (256 q x 512t keys).
The causal diagonal inside the last 512-key window is handled with a
per-core additive mask (host-built data, same shape on every core).

Device layout: host pre-transposes activations to [E, t] and pre-rounds
them to bf16 (the on-device compute dtype -- numerically identical to a
cast-on-load, but half the HBM traffic). Projections are weights-
stationary (lhsT = W e-tiles) producing qT/kT [64, t] bf16; V is projected
activation-stationary producing v [keys, 64] plus a ones column. Attention
computes P^T = [keys, q] scores, exp on ScalarE (one op per key-tile
pair), and [v | 1]^T-weighted accumulation giving (O^T, denom) in one PSUM
group. Host divides by denom and untransposes. bf16 compute, f32
accumulate.

TRN2 instructions may carry at most one sync wait; Bacc.compile()'s
generate_event_semaphores() legalizes multi-wait instructions, but walrus
still rejects >1 wait on DMAs and DVE TensorTensors, so: staged chunks use
fresh SBUF slots (never recycled; full bf16 staging fits at ~193KB per
partition), and the causal mask is accumulated on PE (psum += I.T @ mask)
instead of a DVE add. Projections are interleaved with the attention tiers
they unblock so ScalarE exp overlaps PE projection matmuls.
"""

import ml_dtypes
import numpy as np

import concourse.mybir as mybir
import concourse.tile as tile
from concourse import bacc
from concourse.bass_utils import run_bass_kernel_spmd

B, T, E, D = 4, 4096, 1024, 64
NCORES = 8
NT = 8          # tiers per core
QC = 256        # queries per tier per core
TQ = NT * QC    # 2048 query columns per core
KT = 128        # key tile
EI = E // 128   # 8 e-tiles
CH = 512        # projection chunk (columns per DMA/matmul group)

F32 = mybir.dt.float32
BF16 = mybir.dt.bfloat16

_CACHE = {}


def _build_nc():
    nc = bacc.Bacc()
    qt_d = nc.declare_dram_parameter("qt", [E, TQ], BF16, isOutput=False)
    kt_d = nc.declare_dram_parameter("kt", [E, T], BF16, isOutput=False)
    vt_d = nc.declare_dram_parameter("vt", [E, T], BF16, isOutput=False)
    wq_d = nc.declare_dram_parameter("wq", [E, D], F32, isOutput=False)
    wk_d = nc.declare_dram_parameter("wk", [E, D], F32, isOutput=False)
    wv_d = nc.declare_dram_parameter("wv", [E, D], F32, isOutput=False)
    mask_d = nc.declare_dram_parameter("mask", [4 * KT, QC], F32, isOutput=False)
    id_d = nc.declare_dram_parameter("ident", [128, 128], BF16, isOutput=False)
    out_d = nc.declare_dram_parameter("out", [D + 1, TQ], F32, isOutput=True)

    with tile.TileContext(nc) as tc:
        with (
            tc.tile_pool(name="w", bufs=1) as wpool,
            tc.tile_pool(name="res", bufs=1) as res,
            tc.tile_pool(name="stage", bufs=1) as stage,
            tc.tile_pool(name="pexp", bufs=6) as pexp_pool,
            tc.tile_pool(name="ps_proj", bufs=2, space="PSUM") as ps_proj,
            tc.tile_pool(name="ps_s", bufs=4, space="PSUM") as ps_s,
            tc.tile_pool(name="ps_o", bufs=2, space="PSUM") as ps_o,
        ):
            # --- constants / weights ---
            wq_sb = wpool.tile([128, EI, D], BF16, tag="wq")
            wk_sb = wpool.tile([128, EI, D], BF16, tag="wk")
            wv_sb = wpool.tile([128, EI, D], BF16, tag="wv")
            for w_sb, w_d in ((wq_sb, wq_d), (wk_sb, wk_d), (wv_sb, wv_d)):
                w_raw = wpool.tile([128, EI, D], F32, tag=f"{w_d.name}_raw")
                nc.sync.dma_start(
                    out=w_raw[:], in_=w_d.rearrange("(i p) d -> p i d", p=128)
                )
                nc.vector.tensor_copy(w_sb[:], w_raw[:])
            mask_sb = wpool.tile([128, 4, QC], BF16, tag="mask")
            nc.gpsimd.dma_start(
                out=mask_sb[:], in_=mask_d.rearrange("(w p) q -> p w q", p=128)
            )
            ident = wpool.tile([128, 128], BF16, tag="ident")
            nc.sync.dma_start(out=ident[:], in_=id_d[:])

            kT_sb = res.tile([64, T], BF16, tag="kT")
            qT_sb = res.tile([64, TQ], BF16, tag="qT")
            v_sb = res.tile([128, T // KT, D + 1], BF16, tag="v")
            o_sb = res.tile([D + 1, TQ], F32, tag="o")
            nc.vector.memset(v_sb[:, :, D : D + 1], 1.0)

            def load_chunk(src_d, name, c):
                """DMA one [128, EI, CH] bf16 staging chunk into a fresh
                (never recycled) slot -- recycled slots would need >1 sync
                wait on the DMA, which walrus's DIRECT2D encoding rejects.
                Two half-chunk DMAs let the first projection matmuls start
                before the whole chunk lands."""
                raw = stage.tile([128, EI, CH], BF16, tag=f"{name}{c}")
                rsrc = src_d.rearrange("(i p) t -> p i t", p=128)
                for hh in (0, 1):
                    nc.sync.dma_start(
                        out=raw[:, hh * (EI // 2) : (hh + 1) * (EI // 2), :],
                        in_=rsrc[
                            :, hh * (EI // 2) : (hh + 1) * (EI // 2),
                            c * CH : (c + 1) * CH
                        ],
                    )
                return raw

            def proj_stream(dst_sb, src_d, name, w_sb, c, scale):
                """dst[:, 512c:+512] = scale * (W.T @ X)[:, chunk]."""
                raw = load_chunk(src_d, name, c)
                ps = ps_proj.tile([64, CH], F32, tag="ps")
                for i in range(EI):
                    nc.tensor.matmul(
                        ps[:],
                        lhsT=w_sb[:, i, :],
                        rhs=raw[:, i, :],
                        start=(i == 0),
                        stop=(i == EI - 1),
                    )
                if scale == 1.0:
                    nc.vector.tensor_copy(dst_sb[:, c * CH : (c + 1) * CH], ps[:])
                else:
                    nc.vector.tensor_scalar_mul(
                        dst_sb[:, c * CH : (c + 1) * CH], ps[:], scale
                    )

            def proj_v(c):
                """v[keys 512c:+512, :64] = (V_chunk^T W)  (activation-stationary)."""
                raw = load_chunk(vt_d, "v", c)
                for tt in range(CH // KT):
                    ps = ps_proj.tile([128, D], F32, tag="ps")
                    for i in range(EI):
                        nc.tensor.matmul(
                            ps[:],
                            lhsT=raw[:, i, tt * KT : (tt + 1) * KT],
                            rhs=wv_sb[:, i, :],
                            start=(i == 0),
                            stop=(i == EI - 1),
                        )
                    nc.vector.tensor_copy(
                        v_sb[:, c * (CH // KT) + tt, :D], ps[:]
                    )

            def tier(t):
                """Attention instance t: 256 queries vs 512*t keys, processed
                as key-tile pairs so each exp covers [128, 512]."""
                q0 = (t - 1) * QC
                nk = 4 * t  # key tiles in this tier
                pso = ps_o.tile([D + 1, QC], F32)
                for pj in range(nk // 2):
                    pss = ps_s.tile([128, 2, QC], F32)
                    for u in (0, 1):
                        j = 2 * pj + u
                        w = j - (nk - 4)
                        nc.tensor.matmul(
                            pss[:, u, :],
                            lhsT=kT_sb[:, j * KT : (j + 1) * KT],
                            rhs=qT_sb[:, q0 : q0 + QC],
                            start=True,
                            stop=(w < 0),
                        )
                        if w >= 0:
                            # psum += I.T @ mask -- additive mask on PE (a DVE
                            # TensorTensor may carry only one sync wait)
                            nc.tensor.matmul(
                                pss[:, u, :],
                                lhsT=ident[:],
                                rhs=mask_sb[:, w, :],
                                start=False,
                                stop=True,
                            )
                    pe = pexp_pool.tile([128, 2, QC], BF16, tag="pe")
                    nc.scalar.activation(
                        pe[:], pss[:], mybir.ActivationFunctionType.Exp
                    )
                    for u in (0, 1):
                        j = 2 * pj + u
                        nc.tensor.matmul(
                            pso[:],
                            lhsT=v_sb[:, j, :],
                            rhs=pe[:, u, :],
                            start=(j == 0),
                            stop=(j == nk - 1),
                        )
                nc.vector.tensor_copy(o_sb[:, q0 : q0 + QC], pso[:])

            # --- projections interleaved with the tiers they unblock:
            # tier t needs kT/v prefix chunks 0..t-1 and q chunk (t-1)//2, so
            # after projecting chunk c we can emit tier c+1, letting its exp
            # (ACT) overlap the next chunk's projection matmuls (PE).
            for c in range(T // CH):
                proj_stream(kT_sb, kt_d, "k", wk_sb, c, 1.0)
                proj_v(c)
                if c < TQ // CH:
                    proj_stream(qT_sb, qt_d, "q", wq_sb, c, 0.125)
                tier(c + 1)

            nc.sync.dma_start(out=out_d[:], in_=o_sb[:])

    nc.compile()
    return nc


def _host_shards(K, Q, V, Wk, Wq, Wv):
    in_maps = []
    for c in range(NCORES):
        b, h = c // 2, c % 2
        qt = np.concatenate(
            [
                Q[b, (t - 1) * 512 + h * QC : (t - 1) * 512 + h * QC + QC, :].T
                for t in range(1, NT + 1)
            ],
            axis=1,
        )
        mask = np.where(
            np.arange(4 * KT)[:, None] <= (h * QC + np.arange(QC))[None, :],
            np.float32(0.0),
            np.float32(-1e9),
        ).astype(np.float32)
        in_maps.append(
            {
                "qt": np.ascontiguousarray(qt).astype(ml_dtypes.bfloat16),
                "kt": np.ascontiguousarray(K[b].T).astype(ml_dtypes.bfloat16),
                "vt": np.ascontiguousarray(V[b].T).astype(ml_dtypes.bfloat16),
                "wq": np.ascontiguousarray(Wq, dtype=np.float32),
                "wk": np.ascontiguousarray(Wk, dtype=np.float32),
                "wv": np.ascontiguousarray(Wv, dtype=np.float32),
                "mask": mask,
                "ident": np.eye(128, dtype=ml_dtypes.bfloat16),
            }
        )
    return in_maps


def kernel(K, Q, V, Wk, Wq, Wv, _trace=False):
    K = np.asarray(K)
    Q = np.asarray(Q)
    V = np.asarray(V)
    Wk = np.asarray(Wk)
    Wq = np.asarray(Wq)
    Wv = np.asarray(Wv)

    if "nc" not in _CACHE:
        _CACHE["nc"] = _build_nc()
    nc = _CACHE["nc"]

    in_maps = _host_shards(K, Q, V, Wk, Wq, Wv)
    res = run_bass_kernel_spmd(
        nc, in_maps, core_ids=list(range(NCORES)), trace=_trace
    )
    _CACHE["last_result"] = res

    out = np.empty((B, T, D), dtype=np.float32)
    for c in range(NCORES):
        b, h = c // 2, c % 2
        oc = res.results[c]["out"]  # [65, 2048]
        for t in range(1, NT + 1):
            blk = oc[:, (t - 1) * QC : t * QC]
            qs = (t - 1) * 512 + h * QC
            out[b, qs : qs + QC, :] = (blk[:D, :] / blk[D : D + 1, :]).T
    return out


# revision 15
# speedup vs baseline: 1.0563x; 1.0563x over previous
"""Causal single-head attention on 8 TRN2 NeuronCores.

Problem: K,Q,V [4, 4096, 1024] f32, Wk/Wq/Wv [1024, 64] f32.
out[b,q,:] = softmax_causal((Q Wq)(K Wk)^T / 8) @ (V Wv)

Sharding: core c = 2b+h owns batch b = c//2, half h = c%2. Each batch's
4096 queries are split into 8 tiers of 512; tier t (1-based) attends to a
key prefix rounded up to 512*t. Each tier's 512 queries are split in half
(256 each) across the batch's two cores, so every core executes an
IDENTICAL instruction stream: 8 attention instances (256 q x 512t keys).
The causal diagonal inside the last 512-key window is handled with a
per-core additive mask (host-built data, same shape on every core).

Device layout: host pre-transposes activations to [E, t] and pre-rounds
them to bf16 (the on-device compute dtype -- numerically identical to a
cast-on-load, but half the HBM traffic). Projections are weights-
stationary (lhsT = W e-tiles) producing qT/kT [64, t] bf16; V is projected
activation-stationary producing v [keys, 64] plus a ones column. Attention
computes P^T = [keys, q] scores, exp on ScalarE (one op per key-tile
pair), and [v | 1]^T-weighted accumulation giving (O^T, denom) in one PSUM
group. Host divides by denom and untransposes. bf16 compute, f32
accumulate.

TRN2 instructions may carry at most one sync wait; Bacc.compile()'s
generate_event_semaphores() legalizes multi-wait instructions, but walrus
still rejects >1 wait on DMAs and DVE TensorTensors, so: staged chunks use
fresh SBUF slots (never recycled; full bf16 staging fits at ~193KB per
partition), and the causal mask is accumulated on PE (psum += I.T @ mask)
instead of a DVE add. Projections are interleaved with the attention tiers
they unblock so ScalarE exp overlaps PE projection matmuls.
"""

import ml_dtypes
import numpy as np

import concourse.mybir as mybir
import concourse.tile as tile
from concourse import bacc
from concourse.bass_utils import run_bass_kernel_spmd

B, T, E, D = 4, 4096, 1024, 64
NCORES = 8
NT = 8          # tiers per core
QC = 256        # queries per tier per core
TQ = NT * QC    # 2048 query columns per core
KT = 128        # key tile
EI = E // 128   # 8 e-tiles
CH = 512        # projection chunk (columns per DMA/matmul group)

F32 = mybir.dt.float32
BF16 = mybir.dt.bfloat16

_CACHE = {}


def _build_nc():
    nc = bacc.Bacc()
    qt_d = nc.declare_dram_parameter("qt", [E, TQ], BF16, isOutput=False)
    kt_d = nc.declare_dram_parameter("kt", [E, T], BF16, isOutput=False)
    vt_d = nc.declare_dram_parameter("vt", [E, T], BF16, isOutput=False)
    wq_d = nc.declare_dram_parameter("wq", [E, D], BF16, isOutput=False)
    wk_d = nc.declare_dram_parameter("wk", [E, D], BF16, isOutput=False)
    wv_d = nc.declare_dram_parameter("wv", [E, D], BF16, isOutput=False)
    mask_d = nc.declare_dram_parameter("mask", [4 * KT, QC], F32, isOutput=False)
    id_d = nc.declare_dram_parameter("ident", [128, 128], BF16, isOutput=False)
    out_d = nc.declare_dram_parameter("out", [D + 1, TQ], F32, isOutput=True)

    with tile.TileContext(nc) as tc:
        with (
            tc.tile_pool(name="w", bufs=1) as wpool,
            tc.tile_pool(name="res", bufs=1) as res,
            tc.tile_pool(name="stage", bufs=1) as stage,
            tc.tile_pool(name="pexp", bufs=6) as pexp_pool,
            tc.tile_pool(name="ps_proj", bufs=2, space="PSUM") as ps_proj,
            tc.tile_pool(name="ps_s", bufs=4, space="PSUM") as ps_s,
            tc.tile_pool(name="ps_o", bufs=2, space="PSUM") as ps_o,
        ):
            # --- constants / weights ---
            wq_sb = wpool.tile([128, EI, D], BF16, tag="wq")
            wk_sb = wpool.tile([128, EI, D], BF16, tag="wk")
            wv_sb = wpool.tile([128, EI, D], BF16, tag="wv")
            for w_sb, w_d in ((wq_sb, wq_d), (wk_sb, wk_d), (wv_sb, wv_d)):
                nc.sync.dma_start(
                    out=w_sb[:], in_=w_d.rearrange("(i p) d -> p i d", p=128)
                )
            mask_sb = wpool.tile([128, 4, QC], BF16, tag="mask")
            nc.gpsimd.dma_start(
                out=mask_sb[:], in_=mask_d.rearrange("(w p) q -> p w q", p=128)
            )
            ident = wpool.tile([128, 128], BF16, tag="ident")
            nc.sync.dma_start(out=ident[:], in_=id_d[:])

            kT_sb = res.tile([64, T], BF16, tag="kT")
            qT_sb = res.tile([64, TQ], BF16, tag="qT")
            v_sb = res.tile([128, T // KT, D + 1], BF16, tag="v")
            o_sb = res.tile([D + 1, TQ], F32, tag="o")
            nc.vector.memset(v_sb[:, :, D : D + 1], 1.0)

            def load_chunk(src_d, name, c):
                """DMA one [128, EI, CH] bf16 staging chunk into a fresh
                (never recycled) slot -- recycled slots would need >1 sync
                wait on the DMA, which walrus's DIRECT2D encoding rejects.
                Two half-chunk DMAs let the first projection matmuls start
                before the whole chunk lands."""
                raw = stage.tile([128, EI, CH], BF16, tag=f"{name}{c}")
                rsrc = src_d.rearrange("(i p) t -> p i t", p=128)
                for hh in (0, 1):
                    nc.sync.dma_start(
                        out=raw[:, hh * (EI // 2) : (hh + 1) * (EI // 2), :],
                        in_=rsrc[
                            :, hh * (EI // 2) : (hh + 1) * (EI // 2),
                            c * CH : (c + 1) * CH
                        ],
                    )
                return raw

            def proj_stream(dst_sb, src_d, name, w_sb, c, scale):
                """dst[:, 512c:+512] = scale * (W.T @ X)[:, chunk]."""
                raw = load_chunk(src_d, name, c)
                ps = ps_proj.tile([64, CH], F32, tag="ps")
                for i in range(EI):
                    nc.tensor.matmul(
                        ps[:],
                        lhsT=w_sb[:, i, :],
                        rhs=raw[:, i, :],
                        start=(i == 0),
                        stop=(i == EI - 1),
                    )
                if scale == 1.0:
                    nc.vector.tensor_copy(dst_sb[:, c * CH : (c + 1) * CH], ps[:])
                else:
                    nc.vector.tensor_scalar_mul(
                        dst_sb[:, c * CH : (c + 1) * CH], ps[:], scale
                    )

            def proj_v(c):
                """v[keys 512c:+512, :64] = (V_chunk^T W)  (activation-stationary)."""
                raw = load_chunk(vt_d, "v", c)
                for tt in range(CH // KT):
                    ps = ps_proj.tile([128, D], F32, tag="ps")
                    for i in range(EI):
                        nc.tensor.matmul(
                            ps[:],
                            lhsT=raw[:, i, tt * KT : (tt + 1) * KT],
                            rhs=wv_sb[:, i, :],
                            start=(i == 0),
                            stop=(i == EI - 1),
                        )
                    nc.vector.tensor_copy(
                        v_sb[:, c * (CH // KT) + tt, :D], ps[:]
                    )

            def wave(t, c2):
                """Tier t's 4 key tiles from key chunk c2 (keys [512*c2:+512]):
                scores + exp + AV into a per-wave psum accumulator, then
                folded into o_sb by DVE (copy for the first wave, add after).
                Streaming waves lets every tier's early chunks run as soon as
                they are projected, so the last tier isn't bunched at the
                kernel tail."""
                q0 = (t - 1) * QC
                pso = ps_o.tile([D + 1, QC], F32)
                for pj in (0, 1):
                    pss = ps_s.tile([128, 2, QC], F32)
                    for u in (0, 1):
                        j = 4 * c2 + 2 * pj + u
                        w = j - (4 * t - 4)
                        nc.tensor.matmul(
                            pss[:, u, :],
                            lhsT=kT_sb[:, j * KT : (j + 1) * KT],
                            rhs=qT_sb[:, q0 : q0 + QC],
                            start=True,
                            stop=(w < 0),
                        )
                        if w >= 0:
                            # psum += I.T @ mask -- additive mask on PE (a DVE
                            # TensorTensor may carry only one sync wait)
                            nc.tensor.matmul(
                                pss[:, u, :],
                                lhsT=ident[:],
                                rhs=mask_sb[:, w, :],
                                start=False,
                                stop=True,
                            )
                    pe = pexp_pool.tile([128, 2, QC], BF16, tag="pe")
                    nc.scalar.activation(
                        pe[:], pss[:], mybir.ActivationFunctionType.Exp
                    )
                    for u in (0, 1):
                        j = 4 * c2 + 2 * pj + u
                        nc.tensor.matmul(
                            pso[:],
                            lhsT=v_sb[:, j, :],
                            rhs=pe[:, u, :],
                            start=(pj == 0 and u == 0),
                            stop=(pj == 1 and u == 1),
                        )
                osl = o_sb[:, q0 : q0 + QC]
                if c2 == 0:
                    nc.vector.tensor_copy(osl, pso[:])
                else:
                    nc.vector.tensor_add(osl, osl, pso[:])

            # --- projections interleaved with attention waves. A wave
            # (t, c2) needs kT/v chunk c2 and q chunk (t-1)//2; emit each
            # tier's waves as soon as both are projected.
            emitted = [0] * (NT + 1)
            for c in range(T // CH):
                proj_stream(kT_sb, kt_d, "k", wk_sb, c, 1.0)
                proj_v(c)
                if c < TQ // CH:
                    proj_stream(qT_sb, qt_d, "q", wq_sb, c, 0.125)
                for t in range(1, NT + 1):
                    if (t - 1) // 2 <= c:
                        hi = min(c, t - 1)
                        while emitted[t] <= hi:
                            wave(t, emitted[t])
                            emitted[t] += 1

            nc.sync.dma_start(out=out_d[:], in_=o_sb[:])

    nc.compile()
    return nc


def _host_shards(K, Q, V, Wk, Wq, Wv):
    in_maps = []
    for c in range(NCORES):
        b, h = c // 2, c % 2
        qt = np.concatenate(
            [
                Q[b, (t - 1) * 512 + h * QC : (t - 1) * 512 + h * QC + QC, :].T
                for t in range(1, NT + 1)
            ],
            axis=1,
        )
        mask = np.where(
            np.arange(4 * KT)[:, None] <= (h * QC + np.arange(QC))[None, :],
            np.float32(0.0),
            np.float32(-1e9),
        ).astype(np.float32)
        in_maps.append(
            {
                "qt": np.ascontiguousarray(qt).astype(ml_dtypes.bfloat16),
                "kt": np.ascontiguousarray(K[b].T).astype(ml_dtypes.bfloat16),
                "vt": np.ascontiguousarray(V[b].T).astype(ml_dtypes.bfloat16),
                "wq": np.ascontiguousarray(Wq).astype(ml_dtypes.bfloat16),
                "wk": np.ascontiguousarray(Wk).astype(ml_dtypes.bfloat16),
                "wv": np.ascontiguousarray(Wv).astype(ml_dtypes.bfloat16),
                "mask": mask,
                "ident": np.eye(128, dtype=ml_dtypes.bfloat16),
            }
        )
    return in_maps


def kernel(K, Q, V, Wk, Wq, Wv, _trace=False):
    K = np.asarray(K)
    Q = np.asarray(Q)
    V = np.asarray(V)
    Wk = np.asarray(Wk)
    Wq = np.asarray(Wq)
    Wv = np.asarray(Wv)

    if "nc" not in _CACHE:
        _CACHE["nc"] = _build_nc()
    nc = _CACHE["nc"]

    in_maps = _host_shards(K, Q, V, Wk, Wq, Wv)
    res = run_bass_kernel_spmd(
        nc, in_maps, core_ids=list(range(NCORES)), trace=_trace
    )
    _CACHE["last_result"] = res

    out = np.empty((B, T, D), dtype=np.float32)
    for c in range(NCORES):
        b, h = c // 2, c % 2
        oc = res.results[c]["out"]  # [65, 2048]
        for t in range(1, NT + 1):
            blk = oc[:, (t - 1) * QC : t * QC]
            qs = (t - 1) * 512 + h * QC
            out[b, qs : qs + QC, :] = (blk[:D, :] / blk[D : D + 1, :]).T
    return out


# revision 18
# speedup vs baseline: 1.0602x; 1.0037x over previous
"""Causal single-head attention on 8 TRN2 NeuronCores.

Problem: K,Q,V [4, 4096, 1024] f32, Wk/Wq/Wv [1024, 64] f32.
out[b,q,:] = softmax_causal((Q Wq)(K Wk)^T / 8) @ (V Wv)

Sharding: core c = 2b+h owns batch b = c//2, half h = c%2. Each batch's
4096 queries are split into 8 tiers of 512; tier t (1-based) attends to a
key prefix rounded up to 512*t. Each tier's 512 queries are split in half
(256 each) across the batch's two cores, so every core executes an
IDENTICAL instruction stream: 8 attention instances (256 q x 512t keys).
The causal diagonal inside the last 512-key window is handled with a
per-core additive mask (host-built data, same shape on every core).

Device layout: host pre-transposes activations to [E, t] and pre-rounds
them to bf16 (the on-device compute dtype -- numerically identical to a
cast-on-load, but half the HBM traffic). Projections are weights-
stationary (lhsT = W e-tiles) producing qT/kT [64, t] bf16; V is projected
activation-stationary producing v [keys, 64] plus a ones column. Attention
computes P^T = [keys, q] scores, exp on ScalarE (one op per key-tile
pair), and [v | 1]^T-weighted accumulation giving (O^T, denom) in one PSUM
group. Host divides by denom and untransposes. bf16 compute, f32
accumulate.

TRN2 instructions may carry at most one sync wait; Bacc.compile()'s
generate_event_semaphores() legalizes multi-wait instructions, but walrus
still rejects >1 wait on DMAs and DVE TensorTensors, so: staged chunks use
fresh SBUF slots (never recycled; full bf16 staging fits at ~193KB per
partition), and the causal mask is accumulated on PE (psum += I.T @ mask)
instead of a DVE add. Projections are interleaved with the attention tiers
they unblock so ScalarE exp overlaps PE projection matmuls.
"""

import ml_dtypes
import numpy as np

import concourse.mybir as mybir
import concourse.tile as tile
from concourse import bacc
from concourse.bass_utils import run_bass_kernel_spmd

B, T, E, D = 4, 4096, 1024, 64
NCORES = 8
NT = 8          # tiers per core
QC = 256        # queries per tier per core
TQ = NT * QC    # 2048 query columns per core
KT = 128        # key tile
EI = E // 128   # 8 e-tiles
CH = 512        # projection chunk (columns per DMA/matmul group)

F32 = mybir.dt.float32
BF16 = mybir.dt.bfloat16

_CACHE = {}


def _build_nc():
    nc = bacc.Bacc()
    qt_d = nc.declare_dram_parameter("qt", [E, TQ], BF16, isOutput=False)
    kt_d = nc.declare_dram_parameter("kt", [E, T], BF16, isOutput=False)
    vt_d = nc.declare_dram_parameter("vt", [E, T], BF16, isOutput=False)
    wq_d = nc.declare_dram_parameter("wq", [E, D], BF16, isOutput=False)
    wk_d = nc.declare_dram_parameter("wk", [E, D], BF16, isOutput=False)
    wv_d = nc.declare_dram_parameter("wv", [E, D], BF16, isOutput=False)
    mask_d = nc.declare_dram_parameter("mask", [4 * KT, QC], F32, isOutput=False)
    id_d = nc.declare_dram_parameter("ident", [128, 128], BF16, isOutput=False)
    out_d = nc.declare_dram_parameter("out", [D + 1, TQ], F32, isOutput=True)

    with tile.TileContext(nc) as tc:
        with (
            tc.tile_pool(name="w", bufs=1) as wpool,
            tc.tile_pool(name="res", bufs=1) as res,
            tc.tile_pool(name="stage", bufs=1) as stage,
            tc.tile_pool(name="pexp", bufs=6) as pexp_pool,
            tc.tile_pool(name="ps_proj", bufs=2, space="PSUM") as ps_proj,
            tc.tile_pool(name="ps_s", bufs=4, space="PSUM") as ps_s,
            tc.tile_pool(name="ps_o", bufs=2, space="PSUM") as ps_o,
        ):
            # --- tiles ---
            wq_sb = wpool.tile([128, EI, D], BF16, tag="wq")
            wk_sb = wpool.tile([128, EI, D], BF16, tag="wk")
            wv_sb = wpool.tile([128, EI, D], BF16, tag="wv")
            mask_sb = wpool.tile([128, 4, QC], BF16, tag="mask")
            ident = wpool.tile([128, 128], BF16, tag="ident")

            kT_sb = res.tile([64, T], BF16, tag="kT")
            qT_sb = res.tile([64, TQ], BF16, tag="qT")
            v_sb = res.tile([128, T // KT, D + 1], BF16, tag="v")
            o_sb = res.tile([D + 1, TQ], F32, tag="o")
            nc.vector.memset(v_sb[:, :, D : D + 1], 1.0)

            preloads = {}

            def load_chunk(src_d, name, c, splits=2):
                """DMA one [128, EI, CH] bf16 staging chunk into a fresh
                (never recycled) slot -- recycled slots would need >1 sync
                wait on the DMA, which walrus's DIRECT2D encoding rejects.
                Split sub-DMAs let the first projection matmuls start before
                the whole chunk lands."""
                if (name, c) in preloads:
                    return preloads.pop((name, c))
                raw = stage.tile([128, EI, CH], BF16, tag=f"{name}{c}")
                rsrc = src_d.rearrange("(i p) t -> p i t", p=128)
                step = EI // splits
                for hh in range(splits):
                    nc.sync.dma_start(
                        out=raw[:, hh * step : (hh + 1) * step, :],
                        in_=rsrc[
                            :, hh * step : (hh + 1) * step,
                            c * CH : (c + 1) * CH
                        ],
                    )
                return raw

            def proj_stream(dst_sb, src_d, name, w_sb, c, scale):
                """dst[:, 512c:+512] = scale * (W.T @ X)[:, chunk]."""
                raw = load_chunk(src_d, name, c)
                ps = ps_proj.tile([64, CH], F32, tag="ps")
                for i in range(EI):
                    nc.tensor.matmul(
                        ps[:],
                        lhsT=w_sb[:, i, :],
                        rhs=raw[:, i, :],
                        start=(i == 0),
                        stop=(i == EI - 1),
                    )
                if scale == 1.0:
                    nc.vector.tensor_copy(dst_sb[:, c * CH : (c + 1) * CH], ps[:])
                else:
                    nc.vector.tensor_scalar_mul(
                        dst_sb[:, c * CH : (c + 1) * CH], ps[:], scale
                    )

            def proj_v(c):
                """v[keys 512c:+512, :64] = (V_chunk^T W)  (activation-stationary)."""
                raw = load_chunk(vt_d, "v", c)
                for tt in range(CH // KT):
                    ps = ps_proj.tile([128, D], F32, tag="ps")
                    for i in range(EI):
                        nc.tensor.matmul(
                            ps[:],
                            lhsT=raw[:, i, tt * KT : (tt + 1) * KT],
                            rhs=wv_sb[:, i, :],
                            start=(i == 0),
                            stop=(i == EI - 1),
                        )
                    nc.vector.tensor_copy(
                        v_sb[:, c * (CH // KT) + tt, :D], ps[:]
                    )

            def wave(t, c2):
                """Tier t's 4 key tiles from key chunk c2 (keys [512*c2:+512]):
                scores + exp + AV into a per-wave psum accumulator, then
                folded into o_sb by DVE (copy for the first wave, add after).
                Streaming waves lets every tier's early chunks run as soon as
                they are projected, so the last tier isn't bunched at the
                kernel tail."""
                q0 = (t - 1) * QC
                pso = ps_o.tile([D + 1, QC], F32)
                for pj in (0, 1):
                    pss = ps_s.tile([128, 2, QC], F32)
                    for u in (0, 1):
                        j = 4 * c2 + 2 * pj + u
                        w = j - (4 * t - 4)
                        nc.tensor.matmul(
                            pss[:, u, :],
                            lhsT=kT_sb[:, j * KT : (j + 1) * KT],
                            rhs=qT_sb[:, q0 : q0 + QC],
                            start=True,
                            stop=(w < 0),
                        )
                        if w >= 0:
                            # psum += I.T @ mask -- additive mask on PE (a DVE
                            # TensorTensor may carry only one sync wait; each
                            # half's group must close before the next opens)
                            nc.tensor.matmul(
                                pss[:, u, :],
                                lhsT=ident[:],
                                rhs=mask_sb[:, w, :],
                                start=False,
                                stop=True,
                            )
                    pe = pexp_pool.tile([128, 2, QC], BF16, tag="pe")
                    nc.scalar.activation(
                        pe[:], pss[:], mybir.ActivationFunctionType.Exp
                    )
                    for u in (0, 1):
                        j = 4 * c2 + 2 * pj + u
                        nc.tensor.matmul(
                            pso[:],
                            lhsT=v_sb[:, j, :],
                            rhs=pe[:, u, :],
                            start=(pj == 0 and u == 0),
                            stop=(pj == 1 and u == 1),
                        )
                osl = o_sb[:, q0 : q0 + QC]
                if c2 == 0:
                    nc.vector.tensor_copy(osl, pso[:])
                else:
                    nc.vector.tensor_add(osl, osl, pso[:])
                if c2 == t - 1:
                    nc.sync.dma_start(
                        out=out_d[:, q0 : q0 + QC], in_=osl
                    )

            # --- startup: the tiny weight DMAs go first (they gate the
            # first matmuls), then finely-split chunk-0 data, then the other
            # small constants.
            nc.sync.dma_start(
                out=wk_sb[:], in_=wk_d.rearrange("(i p) d -> p i d", p=128)
            )
            nc.sync.dma_start(
                out=wv_sb[:], in_=wv_d.rearrange("(i p) d -> p i d", p=128)
            )
            nc.sync.dma_start(
                out=wq_sb[:], in_=wq_d.rearrange("(i p) d -> p i d", p=128)
            )
            pre_k0 = load_chunk(kt_d, "k", 0, splits=4)
            pre_v0 = load_chunk(vt_d, "v", 0, splits=4)
            pre_q0 = load_chunk(qt_d, "q", 0, splits=4)
            nc.gpsimd.dma_start(
                out=mask_sb[:], in_=mask_d.rearrange("(w p) q -> p w q", p=128)
            )
            nc.sync.dma_start(out=ident[:], in_=id_d[:])
            preloads.update({("k", 0): pre_k0, ("v", 0): pre_v0, ("q", 0): pre_q0})

            # --- projections interleaved with attention waves. A wave
            # (t, c2) needs kT/v chunk c2 and q chunk (t-1)//2; emit each
            # tier's waves as soon as both are projected.
            emitted = [0] * (NT + 1)
            for c in range(T // CH):
                proj_stream(kT_sb, kt_d, "k", wk_sb, c, 1.0)
                proj_v(c)
                if c < TQ // CH:
                    proj_stream(qT_sb, qt_d, "q", wq_sb, c, 0.125)
                for t in range(1, NT + 1):
                    if (t - 1) // 2 <= c:
                        hi = min(c, t - 1)
                        while emitted[t] <= hi:
                            wave(t, emitted[t])
                            emitted[t] += 1

    nc.compile()
    return nc


def _host_shards(K, Q, V, Wk, Wq, Wv):
    in_maps = []
    for c in range(NCORES):
        b, h = c // 2, c % 2
        qt = np.concatenate(
            [
                Q[b, (t - 1) * 512 + h * QC : (t - 1) * 512 + h * QC + QC, :].T
                for t in range(1, NT + 1)
            ],
            axis=1,
        )
        mask = np.where(
            np.arange(4 * KT)[:, None] <= (h * QC + np.arange(QC))[None, :],
            np.float32(0.0),
            np.float32(-1e9),
        ).astype(np.float32)
        in_maps.append(
            {
                "qt": np.ascontiguousarray(qt).astype(ml_dtypes.bfloat16),
                "kt": np.ascontiguousarray(K[b].T).astype(ml_dtypes.bfloat16),
                "vt": np.ascontiguousarray(V[b].T).astype(ml_dtypes.bfloat16),
                "wq": np.ascontiguousarray(Wq).astype(ml_dtypes.bfloat16),
                "wk": np.ascontiguousarray(Wk).astype(ml_dtypes.bfloat16),
                "wv": np.ascontiguousarray(Wv).astype(ml_dtypes.bfloat16),
                "mask": mask,
                "ident": np.eye(128, dtype=ml_dtypes.bfloat16),
            }
        )
    return in_maps


def kernel(K, Q, V, Wk, Wq, Wv, _trace=False):
    K = np.asarray(K)
    Q = np.asarray(Q)
    V = np.asarray(V)
    Wk = np.asarray(Wk)
    Wq = np.asarray(Wq)
    Wv = np.asarray(Wv)

    if "nc" not in _CACHE:
        _CACHE["nc"] = _build_nc()
    nc = _CACHE["nc"]

    in_maps = _host_shards(K, Q, V, Wk, Wq, Wv)
    res = run_bass_kernel_spmd(
        nc, in_maps, core_ids=list(range(NCORES)), trace=_trace
    )
    _CACHE["last_result"] = res

    out = np.empty((B, T, D), dtype=np.float32)
    for c in range(NCORES):
        b, h = c // 2, c % 2
        oc = res.results[c]["out"]  # [65, 2048]
        for t in range(1, NT + 1):
            blk = oc[:, (t - 1) * QC : t * QC]
            qs = (t - 1) * 512 + h * QC
            out[b, qs : qs + QC, :] = (blk[:D, :] / blk[D : D + 1, :]).T
    return out
